# revision 17
# baseline (speedup 1.0000x reference)
"""Trainium2 Bass kernel for nn_MeanAligning (VQ codebook mean-aligning loss).

Math (see reference):
    count[k] = sum_nhw code[nhw, k]
    num[k,c] = sum_nhw code[nhw, k] * quantized[nhw, c]
    mean     = num / count (count==0 -> mean 0)
    loss     = sum_{k: count>0} ||codebook[k] - mean[k]||^2 / (n_valid * C)

Sharding: K-parallel over the 4096 codebook entries — each of the 8 cores
gets a contiguous 512-column slice of `code` and ALL positions, so each
core owns the *complete* count/num for its K-shard. Only a [1,2] partial
(sum_sq, n_valid) crosses cores at the end (summed on host as the
gather/unshard step).

Device pipeline per core:
  - `code` is staged host-side as fp8e4 (one-hot 0/1 values are exact in
    fp8e4 — a lossless relayout, 4x less HBM traffic than f32) and
    `quant|ones` as fp8e4 (DoubleRow requirement).
  - PSUM-accumulated DoubleRow matmuls (two 128-position tiles per
    matmul): lhsT = [quant|ones|pad] [128, 2, 33], rhs = code
    [128, 2, 512] -> psum acc [33, 512] f32 holding (num^T ; count) —
    exact count, fp32-accumulated num.  Code batches taper off (16..2
    position-tiles) so the PE pipeline drains ~0.2us after the last DMA
    packet instead of a full 1MiB batch's worth of matmuls.  qo is
    streamed in per-batch slices interleaved with code on the opposite
    HWDGE ring; cb last (only needed in the epilogue).
  - Epilogue in a [128,128] layout (all partitions active, ~4x DVE
    speedup vs [32,512]): remap num (4 DVE copies) and count (4 ACT
    copies -> cnt4[4,128]); rcp/valid computed on [4,128] then broadcast
    with ONE PE matmul of the packed [4,256] (rcp|valid) against a
    block-ones lhsT; 4-op DVE chain with a fused multiply+row-sum
    (scalar_tensor_tensor accum_out); GpSimd partition-axis reduces
    produce the final [1,2] in SBUF -> single output DMA.
"""

import os
import sys

import numpy as np

for _p in (
    "/opt/trn_rl_repo",
    "/root/.axon_site",
    "/root/.axon_site/_ro/trn_rl_repo",
):
    if os.path.isdir(_p) and _p not in sys.path:
        sys.path.append(_p)

import concourse.bass as bass  # noqa: E402
import concourse.mybir as mybir  # noqa: E402
import concourse.tile as tile  # noqa: E402
from concourse import bacc, bass_utils  # noqa: E402

F32 = mybir.dt.float32
BF16 = mybir.dt.bfloat16
FP8 = mybir.dt.float8e4
AOT = mybir.AluOpType

# Problem shapes (hardcoded per contract).
N, H, W, C, K = 16, 32, 32, 32, 4096
NHW = N * H * W            # 16384 positions
NCORES = 8
KS = K // NCORES           # 512 codebook entries per core
P = 128                    # partitions
S = NHW // P               # 128 position-tiles
C1 = C + 1                 # 33 = C + ones column
C1P = 33                   # unpadded: DR pair stride = 64*33 = 2112 (16-aligned)
KB = KS // P               # 4 k-groups of 128 per core

# Position-tile batches: big while streaming, tapered at the end so the
# final matmuls (which gate the epilogue) wait on a small final DMA.
BATCHES = [16, 16, 16, 16, 16, 16, 12, 8, 6, 4, 2]
assert sum(BATCHES) == S and all(gb % 2 == 0 for gb in BATCHES)

_CACHE: dict = {}


def _build_nc():
    """Trace + compile the per-core Bass program (identical on all cores)."""
    nc = bacc.Bacc(
        "TRN2",
        target_bir_lowering=False,
        debug=False,
        enable_asserts=False,
        num_devices=NCORES,
    )

    # code_s[p, s*KS + k] = code[s*P + p, k_shard_base + k]  (fp8)
    code_d = nc.dram_tensor("code_s", [P, S * KS], FP8, kind="ExternalInput").ap()
    # qo[p, (a*2+j)*48 + c] = [quant | ones | 0pad][(2a+j)*P + p, c]  (fp8)
    qo_d = nc.dram_tensor("qo", [P, S * C1P], FP8, kind="ExternalInput").ap()
    # cb128[32j+c, x] = codebook[k_shard_base + 128j + x, c]  (f32)
    cb_d = nc.dram_tensor("cb128", [P, P], F32, kind="ExternalInput").ap()
    # cmap[p, m] = 1 if p == 32*(m//32) else 0  (count-broadcast lhsT)
    cmap_d = nc.dram_tensor("cmap", [P, P], BF16, kind="ExternalInput").ap()
    loss_d = nc.dram_tensor("loss", [1, 2], F32, kind="ExternalOutput").ap()

    with tile.TileContext(nc) as tc:
        with (
            tc.tile_pool(name="consts", bufs=1) as consts,
            tc.tile_pool(name="codep", bufs=4) as codep,
            tc.tile_pool(name="work", bufs=1) as work,
            tc.tile_pool(name="acc_psum", bufs=1, space="PSUM") as acc_psum,
            tc.tile_pool(name="aux_psum", bufs=1, space="PSUM") as aux_psum,
        ):
            qo_sb = consts.tile([P, S * C1P], FP8)
            cb_sb = consts.tile([P, P], F32)
            cmap_sb = consts.tile([P, P], BF16)
            # count chunks land on 32-aligned partitions of cntq; the other
            # rows must be zero (they're contracted by the broadcast matmul).
            cntq = consts.tile([P, P], BF16)
            nc.vector.memset(cntq, 0.0)
            ones128 = consts.tile([P, 1], F32)
            nc.vector.memset(ones128, 1.0)

            # first qo slice rides the sync ring ahead of the code stream so
            # matmul 0 unblocks ASAP; the rest goes on the scalar ring.
            q0 = (S * C1P) // 8
            nc.sync.dma_start(qo_sb[:, 0:q0], qo_d[:, 0:q0])
            nc.scalar.dma_start(qo_sb[:, q0:], qo_d[:, q0:])

            # ---- main streaming phase: num^T/count accumulation ----
            acc = acc_psum.tile([C1, KS], F32)  # rows 0..31 = num^T, row 32 = count
            qo3 = qo_sb.rearrange("p (j a c) -> p j a c", a=S // 2, c=C1P)
            n_pairs = S // 2
            off = 0
            for t, gb in enumerate(BATCHES):
                eng = nc.sync if t % 2 == 0 else nc.scalar
                ctile = codep.tile([P, gb * KS], FP8, tag="code")
                if t == 0:
                    # split the first batch across BOTH rings in quarters so
                    # matmul 0 unblocks ASAP.
                    q = gb * KS // 4
                    for i in range(4):
                        e = nc.sync if i % 2 == 0 else nc.scalar
                        e.dma_start(
                            ctile[:, i * q : (i + 1) * q],
                            code_d[:, i * q : (i + 1) * q],
                        )
                else:
                    eng.dma_start(ctile, code_d[:, off * KS : (off + gb) * KS])
                if t == 2:
                    # cb/cmap only gate the epilogue; issue mid-stream.
                    nc.scalar.dma_start(cb_sb, cb_d)
                    nc.scalar.dma_start(cmap_sb, cmap_d)
                ct3 = ctile.rearrange("p (g k) -> p g k", k=KS)
                for b in range(gb // 2):
                    a = off // 2 + b
                    nc.tensor.matmul(
                        acc,
                        qo3[:, :, a, 0:C1],
                        ct3[:, 2 * b : 2 * b + 2, :],
                        start=(a == 0),
                        stop=(a == n_pairs - 1),
                        perf_mode=mybir.MatmulPerfMode.DoubleRow,
                    )
                off += gb

            # ---- epilogue in [128,128] layout ----
            # count chunk j -> cntq row 32j; num -> n128.  Copies split
            # DVE/ACT so the two remaps run in parallel; emitted in intended
            # execution order (count first: it gates the broadcast matmul).
            n128 = work.tile([P, P], F32)
            nc.vector.tensor_copy(cntq[0:1, :], acc[C:C1, 0:P])
            nc.vector.tensor_copy(cntq[32:33, :], acc[C:C1, P : 2 * P])
            nc.scalar.copy(cntq[64:65, :], acc[C:C1, 2 * P : 3 * P])
            nc.scalar.copy(cntq[96:97, :], acc[C:C1, 3 * P : 4 * P])
            nc.vector.tensor_copy(n128[0:32, :], acc[0:C, 0:P])
            nc.vector.tensor_copy(n128[32:64, :], acc[0:C, P : 2 * P])
            nc.scalar.copy(n128[64:96, :], acc[0:C, 2 * P : 3 * P])
            nc.scalar.copy(n128[96:128, :], acc[0:C, 3 * P : 4 * P])
            # broadcast count chunks across the 32 C partitions in ONE matmul:
            # c_ps[32j+c, x] = count[128j + x]
            c_ps = aux_psum.tile([P, P], F32, tag="cb")
            nc.tensor.matmul(c_ps, cmap_sb, cntq, start=True, stop=True)

            fin = work.tile([1, 2], F32)
            stack = work.tile([P, 1], F32)
            safe = work.tile([P, P], F32)
            nc.vector.tensor_scalar_max(safe, c_ps, 0.5)
            rcp = work.tile([P, P], F32)
            nc.vector.reciprocal_approx_fast(rcp, safe)
            # valid mask on DVE (gpsimd can't read PSUM); its grand total on
            # gpsimd (SBUF->SBUF), concurrent with the fp32 (1-port) DVE
            # chain; total lands straight in fin[0,1]
            valid = work.tile([P, P], F32)
            nc.vector.tensor_scalar(valid, c_ps, 0.5, None, AOT.is_gt)
            nc.gpsimd.reduce_sum(
                fin[:, 1:2], valid, axis=mybir.AxisListType.XYZWC
            )
            mean = work.tile([P, P], F32)
            nc.vector.tensor_mul(mean, n128, rcp)
            # e = cb - mean  (fused: (mean * -1) + cb)
            e = work.tile([P, P], F32)
            nc.vector.scalar_tensor_tensor(e, mean, -1.0, cb_sb, AOT.mult, AOT.add)
            dm = work.tile([P, P], F32)
            nc.vector.tensor_mul(dm, e, valid)
            # dm^2 with fused row-sum -> stack
            dsq = work.tile([P, P], F32)
            nc.vector.scalar_tensor_tensor(
                dsq, dm, 1.0, dm, AOT.mult, AOT.mult, accum_out=stack
            )
            # partition reduce [128,1] -> [1,1] on PE
            fin_ps = aux_psum.tile([1, 1], F32, tag="fin")
            nc.tensor.matmul(fin_ps, ones128, stack, start=True, stop=True)
            nc.vector.tensor_copy(fin[:, 0:1], fin_ps)
            nc.sync.dma_start(loss_d, fin)

    nc.compile()
    return nc


def _get_nc():
    if "nc" not in _CACHE:
        _CACHE["nc"] = _build_nc()
    return _CACHE["nc"]


def _make_in_maps(quantized, code, codebook):
    np_fp8 = mybir.dt.np(FP8)

    q2 = np.asarray(quantized, dtype=np.float32).reshape(NHW, C)
    code2 = np.asarray(code, dtype=np.float32).reshape(NHW, K)
    cb = np.asarray(codebook, dtype=np.float32)

    qo = np.zeros((NHW, C1P), np.float32)
    qo[:, 0:C] = q2
    qo[:, C] = 1.0
    # qo_kc[p, ((j*64 + a)*33 + c)] = qo[(2a+j)*128 + p, c]  (j-major blocks
    # so the DoubleRow pair stride is 64*33 = 2112, a multiple of 16)
    qo_kc = np.ascontiguousarray(
        qo.reshape(S // 2, 2, P, C1P).transpose(2, 1, 0, 3)
    ).reshape(P, S * C1P).astype(np_fp8)

    # cmap[p, m] = 1 if p == 32*(m//32) else 0
    cmap = np.zeros((P, P), np.float32)
    for j in range(4):
        cmap[32 * j, 32 * j : 32 * (j + 1)] = 1.0
    cmap = cmap.astype(mybir.dt.np(BF16))

    code8 = code2.astype(np_fp8)  # 0/1 values: exact
    in_maps = []
    for j in range(NCORES):
        ksl = slice(j * KS, (j + 1) * KS)
        # [NHW, KS] -> [S, P, KS] -> [P, S, KS] -> [128, S*KS]
        code_j = np.ascontiguousarray(
            code8[:, ksl].reshape(S, P, KS).swapaxes(0, 1)
        ).reshape(P, S * KS)
        # cb128[32j+c, x] = cb[ksl][128j + x, c]
        cb_j = np.ascontiguousarray(
            cb[ksl].reshape(KB, P, C).transpose(0, 2, 1)
        ).reshape(P, P)
        in_maps.append({"code_s": code_j, "qo": qo_kc, "cb128": cb_j, "cmap": cmap})
    return in_maps


def run(quantized, code, codebook, trace=False, **spmd_kwargs):
    """Run the SPMD kernel; returns (loss_scalar, BassKernelResults)."""
    nc = _get_nc()
    in_maps = _make_in_maps(quantized, code, codebook)
    res = bass_utils.run_bass_kernel_spmd(
        nc, in_maps, core_ids=list(range(NCORES)), trace=trace, **spmd_kwargs
    )
    parts = np.stack(
        [np.asarray(res.results[j]["loss"]).reshape(2) for j in range(NCORES)]
    )
    tot = parts.sum(axis=0, dtype=np.float32)
    # tot[1] = 32 * n_valid (valid mask summed over the broadcast layout)
    loss = np.float32(tot[0] / max(tot[1], np.float32(C)))
    return np.asarray(loss, dtype=np.float32).reshape(()), res


def kernel(quantized, code, codebook):
    loss, _ = run(quantized, code, codebook)
    return loss
